# revision 1
# baseline (speedup 1.0000x reference)
"""Griffin block (Hawk RG-LRU + GatedMLP) Trainium2 Bass kernel.

Sharding: 8 chunks = 4 batches x 2 time-halves, one per NeuronCore.
Per-core layout is feature-major ([channels, tokens]) so that:
  - every projection is lhsT(=W^T tile) @ rhs(=activation tile) on TensorE,
  - the RG-LRU recurrence is a native DVE tensor_tensor_scan along the
    free (time) axis per 128-channel partition tile,
  - the depthwise causal conv is 4 per-partition-scalar FMAs on shifted
    time slices,
  - per-channel gate params ride the activation scale/bias ports.
The cross-half scan dependency is handled with a local scan + pairwise
AllGather of the boundary state + cumulative-alpha fixup (phase 2).
Host does all transposes / weight pre-scaling (rmsnorm gamma folding).
"""

import numpy as np
import ml_dtypes
from contextlib import ExitStack

import concourse.bass as bass
import concourse.bacc as bacc
import concourse.tile as tile
from concourse import mybir
from concourse.bass_utils import run_bass_kernel_spmd

F32 = mybir.dt.float32
BF16 = mybir.dt.bfloat16
AF = mybir.ActivationFunctionType
OP = mybir.AluOpType

D = 1024
NP = 128          # partitions
NCT = D // NP     # channel tiles = 8
KCONV = 4
N_CORES = 8

_BF = ml_dtypes.bfloat16


def build_program(T_core: int, L: int, gelu_approx: bool = False,
                  L2: int | None = None):
    """Emit the SPMD program. T_core tokens per core, token tile L."""
    assert T_core % L == 0
    n_tiles = T_core // L
    if L2 is None:
        L2 = L // 2
    n_tiles2 = T_core // L2
    H2 = 2 * D        # hawk proj width (2048)
    HID = 2 * H2      # gmlp hidden total rows (4096): gate2 [0:2048), v [2048:4096)

    nc = bacc.Bacc("TRN2", target_bir_lowering=False, debug=False,
                   num_devices=N_CORES)

    # ---- DRAM parameters (per-core data via in_maps) ----
    x_d = nc.dram_tensor("x", [D, 3 + T_core], F32, kind="ExternalInput")
    wi_d = nc.dram_tensor("wi", [D, H2], BF16, kind="ExternalInput")      # input_w.T (gamma folded)
    wg_d = nc.dram_tensor("wg", [D, H2], BF16, kind="ExternalInput")      # gates_w.T
    wo_d = nc.dram_tensor("wo", [D, D], BF16, kind="ExternalInput")       # output_w.T
    wgr_d = nc.dram_tensor("wgr", [D, HID], BF16, kind="ExternalInput")   # grow_w.T (gamma folded)
    wsh_d = nc.dram_tensor("wsh", [H2, D], BF16, kind="ExternalInput")    # shrink_w.T
    # per-channel params, laid out [partition, ch_tile]
    msp_d = nc.dram_tensor("msp", [NP, NCT], F32, kind="ExternalInput")    # -8*softplus(fb)
    msp2_d = nc.dram_tensor("msp2", [NP, NCT], F32, kind="ExternalInput")  # 2*msp
    gbf_d = nc.dram_tensor("gbf", [NP, NCT], F32, kind="ExternalInput")    # gates_b[:D]
    gbi_d = nc.dram_tensor("gbi", [NP, NCT], F32, kind="ExternalInput")    # gates_b[D:]
    cw_d = nc.dram_tensor("cw", [NP, KCONV * NCT], F32, kind="ExternalInput")  # conv w taps
    cb_d = nc.dram_tensor("cb", [NP, NCT], F32, kind="ExternalInput")      # conv bias
    cmask_d = nc.dram_tensor("cmask", [NP, 1], F32, kind="ExternalInput")  # 1.0 iff second half

    out_d = nc.dram_tensor("out", [D, T_core], F32, kind="ExternalOutput")

    # ---- internal DRAM scratch ----
    h_d = nc.dram_tensor("h_spill", [D, T_core], F32)
    ac_d = nc.dram_tensor("ac_spill", [D, T_core], BF16)
    g_d = nc.dram_tensor("g_spill", [D, T_core], BF16)
    carry_loc = nc.dram_tensor("carry_loc", [1, D], F32)
    carry_gth = nc.dram_tensor("carry_gth", [2, D], F32)

    with tile.TileContext(nc) as tc, ExitStack() as top:
        # ------- persistent small constants -------
        cpool = top.enter_context(tc.tile_pool(name="consts", bufs=1))
        ones_bf = cpool.tile([NP, NP], BF16, name="ones_bf")
        nc.vector.memset(ones_bf[:], 1.0)
        ones_f = cpool.tile([NP, L], F32, name="ones_f")
        nc.vector.memset(ones_f[:], 1.0)
        msp_sb = cpool.tile([NP, NCT], F32, name="msp_sb")
        nc.sync.dma_start(msp_sb[:], msp_d.ap()[:, :])
        msp2_sb = cpool.tile([NP, NCT], F32, name="msp2_sb")
        nc.sync.dma_start(msp2_sb[:], msp2_d.ap()[:, :])
        gbf_sb = cpool.tile([NP, NCT], F32, name="gbf_sb")
        nc.sync.dma_start(gbf_sb[:], gbf_d.ap()[:, :])
        gbi_sb = cpool.tile([NP, NCT], F32, name="gbi_sb")
        nc.sync.dma_start(gbi_sb[:], gbi_d.ap()[:, :])
        cw_sb = cpool.tile([NP, KCONV * NCT], F32, name="cw_sb")
        nc.sync.dma_start(cw_sb[:], cw_d.ap()[:, :])
        cb_sb = cpool.tile([NP, NCT], F32, name="cb_sb")
        nc.sync.dma_start(cb_sb[:], cb_d.ap()[:, :])
        cmask_sb = cpool.tile([NP, 1], F32, name="cmask_sb")
        nc.sync.dma_start(cmask_sb[:], cmask_d.ap()[:, :])
        hlast = cpool.tile([NP, NCT], F32, name="hlast")
        alast = cpool.tile([NP, NCT], F32, name="alast")
        epsb = cpool.tile([NP, 1], F32, name="epsb")
        nc.vector.memset(epsb[:], 1e-20)
        onepb = cpool.tile([NP, 1], F32, name="onepb")
        nc.vector.memset(onepb[:], 1.0 + 1e-6)

        # =========================== PHASE 1 ===========================
        with ExitStack() as p1:
            wpool = p1.enter_context(tc.tile_pool(name="w1", bufs=1))
            wi_sb = wpool.tile([NP, NCT * H2], BF16, name="wi_sb")
            wg_sb = wpool.tile([NP, NCT * H2], BF16, name="wg_sb")
            for k in range(NCT):
                nc.sync.dma_start(wi_sb[:, k * H2:(k + 1) * H2],
                                  wi_d.ap()[k * NP:(k + 1) * NP, :])
                nc.sync.dma_start(wg_sb[:, k * H2:(k + 1) * H2],
                                  wg_d.ap()[k * NP:(k + 1) * NP, :])

            xp = p1.enter_context(tc.tile_pool(name="xp", bufs=10))
            sp = p1.enter_context(tc.tile_pool(name="sp", bufs=2))
            znp = p1.enter_context(tc.tile_pool(name="znp", bufs=10))
            zp = p1.enter_context(tc.tile_pool(name="zp", bufs=10))
            zcp = p1.enter_context(tc.tile_pool(name="zcp", bufs=3))
            zcbp = p1.enter_context(tc.tile_pool(name="zcbp", bufs=10))
            sfp = p1.enter_context(tc.tile_pool(name="sfp", bufs=9))
            gp = p1.enter_context(tc.tile_pool(name="gp", bufs=3))
            ap_ = p1.enter_context(tc.tile_pool(name="ap", bufs=4))
            hp = p1.enter_context(tc.tile_pool(name="hp", bufs=3))
            zhp = p1.enter_context(tc.tile_pool(name="zhp", bufs=2))
            pmm = p1.enter_context(
                tc.tile_pool(name="pmm", bufs=5, space="PSUM"))
            pssq = p1.enter_context(
                tc.tile_pool(name="pssq", bufs=2, space="PSUM"))

            def norm_tiles(x_tiles, w, tag):
                """rmsnorm scale: s = exp(-0.5*ln(ssq)) = 1/||x||, bcast over
                partitions; returns per-tile bf16 normalized x. ln/exp share
                one ACT table set with the alpha/beta exps."""
                ssq = pssq.tile([NP, w], F32, name=f"ssq_{tag}", tag="ssq")
                for i in range(NCT):
                    xsq = sp.tile([NP, w], BF16, name=f"xsq_{tag}_{i}", tag="xsq")
                    nc.vector.tensor_tensor(xsq[:], x_tiles[i][:], x_tiles[i][:],
                                            OP.mult)
                    nc.tensor.matmul(ssq[:], ones_bf[:], xsq[:],
                                     start=(i == 0), stop=(i == NCT - 1))
                lssq = sp.tile([NP, w], F32, name=f"lssq_{tag}", tag="lssq")
                nc.scalar.activation(lssq[:], ssq[:], AF.Ln, bias=epsb[:, 0:1])
                s = sp.tile([NP, w], F32, name=f"s_{tag}", tag="s")
                nc.scalar.activation(s[:], lssq[:], AF.Exp, scale=-0.5)
                xn = []
                for i in range(NCT):
                    t = znp.tile([NP, w], BF16, name=f"xn_{tag}_{i}", tag="xn")
                    nc.vector.tensor_tensor(t[:], x_tiles[i][:], s[:], OP.mult)
                    xn.append(t)
                return xn

            # ---- halo z: conv inputs for the 3 tokens before this chunk ----
            xh = [xp.tile([NP, 3], F32, name=f"xh_{i}", tag="xh") for i in range(NCT)]
            for i in range(NCT):
                nc.sync.dma_start(xh[i][:], x_d.ap()[i * NP:(i + 1) * NP, 0:3])
            xnh = norm_tiles(xh, 3, "h")
            zhalo_prev = zhp.tile([NP, 3 * NCT], BF16, name="zhalo_h", tag="zhalo")
            for m in range(NCT):  # z half rows of input_w = cols [D + 128m ...)
                ps = pmm.tile([NP, 3], F32, name=f"zh_ps_{m}", tag="mm")
                for k in range(NCT):
                    lhs = wi_sb[:, k * H2 + D + m * NP: k * H2 + D + (m + 1) * NP]
                    nc.tensor.matmul(ps[:], lhs, xnh[k][:],
                                     start=(k == 0), stop=(k == NCT - 1))
                nc.vector.tensor_copy(zhalo_prev[:, 3 * m:3 * m + 3], ps[:])

            # ---- main phase-1 tiles ----
            dmae = [nc.sync, nc.scalar, nc.gpsimd]
            for t in range(n_tiles):
                c0 = 3 + t * L
                x_t = [xp.tile([NP, L], F32, name=f"x_{t}_{i}", tag="x")
                       for i in range(NCT)]
                for i in range(NCT):
                    dmae[i % 3].dma_start(
                        x_t[i][:], x_d.ap()[i * NP:(i + 1) * NP, c0:c0 + L])
                xn = norm_tiles(x_t, L, f"t{t}")

                # input proj, z half FIRST so the conv chain starts early
                z_sb = []
                for m in range(NCT, 2 * NCT):
                    ps = pmm.tile([NP, L], F32, name=f"u_ps_{t}_{m}", tag="mm")
                    for k in range(NCT):
                        lhs = wi_sb[:, k * H2 + m * NP: k * H2 + (m + 1) * NP]
                        nc.tensor.matmul(ps[:], lhs, xn[k][:],
                                         start=(k == 0), stop=(k == NCT - 1))
                    i = m - NCT
                    zt = zp.tile([NP, L + 3], BF16, name=f"z_{t}_{i}", tag="z")
                    nc.gpsimd.tensor_copy(zt[:, 0:3],
                                          zhalo_prev[:, 3 * i:3 * i + 3])
                    nc.vector.tensor_copy(zt[:, 3:3 + L], ps[:])
                    z_sb.append(zt)

                # depthwise causal conv: accumulate 4 taps in f32, cast last
                zhalo_cur = zhp.tile([NP, 3 * NCT], BF16, name=f"zhalo_{t}",
                                     tag="zhalo")
                zcb = []
                for i in range(NCT):
                    zci = zcp.tile([NP, L], F32, name=f"zc_{t}_{i}", tag="zc")
                    nc.scalar.activation(zci[:], z_sb[i][:, 0:L], AF.Identity,
                                         scale=cw_sb[:, 0 * NCT + i:0 * NCT + i + 1],
                                         bias=cb_sb[:, i:i + 1])
                    for k in (1, 2):
                        nc.vector.scalar_tensor_tensor(
                            zci[:], z_sb[i][:, k:k + L],
                            cw_sb[:, k * NCT + i:k * NCT + i + 1],
                            zci[:], op0=OP.mult, op1=OP.add)
                    zcbi = zcbp.tile([NP, L], BF16, name=f"zcb_{t}_{i}", tag="zcb")
                    nc.vector.scalar_tensor_tensor(
                        zcbi[:], z_sb[i][:, 3:3 + L],
                        cw_sb[:, 3 * NCT + i:3 * NCT + i + 1],
                        zci[:], op0=OP.mult, op1=OP.add)
                    nc.gpsimd.tensor_copy(zhalo_cur[:, 3 * i:3 * i + 3],
                                          z_sb[i][:, L:L + 3])
                    zcb.append(zcbi)
                zhalo_prev = zhalo_cur

                # gate half of the input proj (+ gelu) - not needed until
                # phase 2, so emitted after the conv-critical z path
                for m in range(NCT):
                    ps = pmm.tile([NP, L], F32, name=f"u_ps_{t}_{m}", tag="mm")
                    for k in range(NCT):
                        lhs = wi_sb[:, k * H2 + m * NP: k * H2 + (m + 1) * NP]
                        nc.tensor.matmul(ps[:], lhs, xn[k][:],
                                         start=(k == 0), stop=(k == NCT - 1))
                    g_bf = gp.tile([NP, L], BF16, name=f"g_{t}_{m}", tag="g")
                    if gelu_approx:
                        sg_ = sp.tile([NP, L], F32, name=f"sg_{t}_{m}", tag="sg")
                        nc.scalar.activation(sg_[:], ps[:], AF.Sigmoid,
                                             scale=1.702)
                        nc.vector.tensor_tensor(g_bf[:], ps[:], sg_[:], OP.mult)
                    else:
                        nc.scalar.activation(g_bf[:], ps[:], AF.Gelu)
                    dmae[m % 3].dma_start(
                        g_d.ap()[m * NP:(m + 1) * NP, t * L:(t + 1) * L],
                        g_bf[:])

                # gates proj -> batched sigmoids (one ACT table set);
                # PE interleaves f/i pairs so <=4 psum tiles are live
                sf = [None] * NCT
                si = [None] * NCT
                for i in range(NCT):
                    psf = pmm.tile([NP, L], F32, name=f"f_ps_{t}_{i}", tag="mm")
                    for k in range(NCT):
                        lhs = wg_sb[:, k * H2 + i * NP: k * H2 + (i + 1) * NP]
                        nc.tensor.matmul(psf[:], lhs, zcb[k][:],
                                         start=(k == 0), stop=(k == NCT - 1))
                    sfi = sfp.tile([NP, L], BF16, name=f"sf_{t}_{i}", tag="sf")
                    nc.scalar.activation(sfi[:], psf[:], AF.Sigmoid,
                                         bias=gbf_sb[:, i:i + 1])
                    sf[i] = sfi
                    psi = pmm.tile([NP, L], F32, name=f"i_ps_{t}_{i}", tag="mm")
                    for k in range(NCT):
                        lhs = wg_sb[:, k * H2 + D + i * NP: k * H2 + D + (i + 1) * NP]
                        nc.tensor.matmul(psi[:], lhs, zcb[k][:],
                                         start=(k == 0), stop=(k == NCT - 1))
                    sii = sfp.tile([NP, L], BF16, name=f"si_{t}_{i}", tag="si")
                    nc.scalar.activation(sii[:], psi[:], AF.Sigmoid,
                                         bias=gbi_sb[:, i:i + 1])
                    si[i] = sii

                # exp/ln block (single ACT table set): alpha, alpha^2,
                # beta = exp(0.5*ln(1+eps-alpha^2)); then xs and the scans
                for i in range(NCT):
                    alpha = ap_.tile([NP, L], F32, name=f"al_{t}_{i}", tag="alpha")
                    nc.scalar.activation(alpha[:], sf[i][:], AF.Exp,
                                         scale=msp_sb[:, i:i + 1])
                    a2 = sp.tile([NP, L], F32, name=f"a2_{t}_{i}", tag="a2")
                    nc.scalar.activation(a2[:], sf[i][:], AF.Exp,
                                         scale=msp2_sb[:, i:i + 1])
                    u2 = sp.tile([NP, L], F32, name=f"u2_{t}_{i}", tag="u2")
                    nc.vector.tensor_scalar(u2[:], a2[:], -1.0, 1.0 + 1e-6,
                                            op0=OP.mult, op1=OP.add)
                    lu2 = sp.tile([NP, L], F32, name=f"lu2_{t}_{i}", tag="lu2")
                    nc.scalar.activation(lu2[:], u2[:], AF.Ln)
                    beta = sp.tile([NP, L], F32, name=f"be_{t}_{i}", tag="beta")
                    nc.scalar.activation(beta[:], lu2[:], AF.Exp, scale=0.5)

                    sz = sp.tile([NP, L], F32, name=f"sz_{t}_{i}", tag="sz")
                    nc.vector.tensor_tensor(sz[:], si[i][:], zcb[i][:], OP.mult)
                    xs = sp.tile([NP, L], F32, name=f"xs_{t}_{i}", tag="xs")
                    nc.vector.tensor_tensor(xs[:], sz[:], beta[:], OP.mult)

                    h = hp.tile([NP, L], F32, name=f"h_{t}_{i}", tag="h")
                    h_init = 0.0 if t == 0 else hlast[:, i:i + 1]
                    nc.vector.tensor_tensor_scan(h[:], alpha[:], xs[:], h_init,
                                                 op0=OP.mult, op1=OP.add)
                    nc.gpsimd.tensor_copy(hlast[:, i:i + 1], h[:, L - 1:L])
                    dmae[i % 3].dma_start(
                        h_d.ap()[i * NP:(i + 1) * NP, t * L:(t + 1) * L], h[:])
                    if t == n_tiles - 1:
                        nc.scalar.dma_start(carry_loc.ap()[0:1, i * NP:(i + 1) * NP],
                                            h[:, L - 1:L])

                    ac = hp.tile([NP, L], BF16, name=f"ac_{t}_{i}", tag="ac")
                    a_init = 1.0 if t == 0 else alast[:, i:i + 1]
                    nc.vector.tensor_tensor_scan(ac[:], alpha[:], ones_f[:, 0:L],
                                                 a_init, op0=OP.mult, op1=OP.mult)
                    nc.gpsimd.tensor_copy(alast[:, i:i + 1], ac[:, L - 1:L])
                    dmae[(i + 1) % 3].dma_start(
                        ac_d.ap()[i * NP:(i + 1) * NP, t * L:(t + 1) * L], ac[:])

            # ---- pairwise carry exchange ----
            nc.gpsimd.collective_compute(
                "AllGather", OP.bypass,
                replica_groups=[[0, 1], [2, 3], [4, 5], [6, 7]],
                ins=[carry_loc.ap()], outs=[carry_gth.ap()])

        # =========================== PHASE 2 ===========================
        with ExitStack() as p2:
            wpool2 = p2.enter_context(tc.tile_pool(name="w2", bufs=1))
            wo_sb = wpool2.tile([NP, NCT * D], BF16, name="wo_sb")
            wgr_sb = wpool2.tile([NP, NCT * HID], BF16, name="wgr_sb")
            wsh_sb = wpool2.tile([NP, 2 * NCT * D], BF16, name="wsh_sb")
            for k in range(NCT):
                nc.sync.dma_start(wo_sb[:, k * D:(k + 1) * D],
                                  wo_d.ap()[k * NP:(k + 1) * NP, :])
                nc.sync.dma_start(wgr_sb[:, k * HID:(k + 1) * HID],
                                  wgr_d.ap()[k * NP:(k + 1) * NP, :])
            for k in range(2 * NCT):
                nc.sync.dma_start(wsh_sb[:, k * D:(k + 1) * D],
                                  wsh_d.ap()[k * NP:(k + 1) * NP, :])

            cg = cpool.tile([NP, NCT], F32, name="cg")
            for i in range(NCT):
                nc.sync.dma_start(
                    cg[:, i:i + 1],
                    carry_gth.ap()[0:1, i * NP:(i + 1) * NP].rearrange("a c -> c a"))
            carrym = cpool.tile([NP, NCT], F32, name="carrym")
            nc.vector.tensor_scalar(carrym[:], cg[:], cmask_sb[:, 0:1], None,
                                    op0=OP.mult)

            xp2 = p2.enter_context(tc.tile_pool(name="xp2", bufs=10))
            hp2 = p2.enter_context(tc.tile_pool(name="hp2", bufs=3))
            sp2 = p2.enter_context(tc.tile_pool(name="sp2", bufs=3))
            ghp = p2.enter_context(tc.tile_pool(name="ghp", bufs=10))
            x1p = p2.enter_context(tc.tile_pool(name="x1p", bufs=10))
            x1np = p2.enter_context(tc.tile_pool(name="x1np", bufs=10))
            gvp = p2.enter_context(tc.tile_pool(name="gvp", bufs=18))
            op_ = p2.enter_context(tc.tile_pool(name="op", bufs=3))
            pmm2 = p2.enter_context(
                tc.tile_pool(name="pmm2", bufs=6, space="PSUM"))
            pssq2 = p2.enter_context(
                tc.tile_pool(name="pssq2", bufs=2, space="PSUM"))

            for t in range(n_tiles2):
                c0 = 3 + t * L2
                gh = []
                x_t = []
                for i in range(NCT):
                    hpi = hp2.tile([NP, L2], F32, name=f"hr_{t}_{i}", tag="hr")
                    nc.sync.dma_start(
                        hpi[:], h_d.ap()[i * NP:(i + 1) * NP, t * L2:(t + 1) * L2])
                    aci = hp2.tile([NP, L2], BF16, name=f"acr_{t}_{i}", tag="acr")
                    nc.sync.dma_start(
                        aci[:], ac_d.ap()[i * NP:(i + 1) * NP, t * L2:(t + 1) * L2])
                    gi = hp2.tile([NP, L2], BF16, name=f"gr_{t}_{i}", tag="gr")
                    nc.sync.dma_start(
                        gi[:], g_d.ap()[i * NP:(i + 1) * NP, t * L2:(t + 1) * L2])
                    xi = xp2.tile([NP, L2], F32, name=f"x2_{t}_{i}", tag="x2")
                    nc.sync.dma_start(xi[:],
                                      x_d.ap()[i * NP:(i + 1) * NP, c0:c0 + L2])
                    x_t.append(xi)

                    hf = sp2.tile([NP, L2], F32, name=f"hf_{t}_{i}", tag="hf")
                    nc.vector.scalar_tensor_tensor(hf[:], aci[:],
                                                   carrym[:, i:i + 1], hpi[:],
                                                   op0=OP.mult, op1=OP.add)
                    ghi = ghp.tile([NP, L2], BF16, name=f"gh_{t}_{i}", tag="gh")
                    nc.vector.tensor_tensor(ghi[:], gi[:], hf[:], OP.mult)
                    gh.append(ghi)

                # output proj + residual
                x1 = []
                for m in range(NCT):
                    ps = pmm2.tile([NP, L2], F32, name=f"o_ps_{t}_{m}", tag="mm2")
                    for k in range(NCT):
                        lhs = wo_sb[:, k * D + m * NP: k * D + (m + 1) * NP]
                        nc.tensor.matmul(ps[:], lhs, gh[k][:],
                                         start=(k == 0), stop=(k == NCT - 1))
                    x1m = x1p.tile([NP, L2], F32, name=f"x1_{t}_{m}", tag="x1")
                    nc.vector.tensor_tensor(x1m[:], ps[:], x_t[m][:], OP.add)
                    x1.append(x1m)

                # norm2
                ssq = pssq2.tile([NP, L2], F32, name=f"ssq2_{t}", tag="ssq2")
                for i in range(NCT):
                    xsq = sp2.tile([NP, L2], BF16, name=f"x1sq_{t}_{i}", tag="x1sq")
                    nc.scalar.activation(xsq[:], x1[i][:], AF.Square)
                    nc.tensor.matmul(ssq[:], ones_bf[:], xsq[:],
                                     start=(i == 0), stop=(i == NCT - 1))
                sq = sp2.tile([NP, L2], F32, name=f"sq2_{t}", tag="sq2")
                nc.scalar.activation(sq[:], ssq[:], AF.Sqrt, bias=epsb[:, 0:1])
                s2 = sp2.tile([NP, L2], F32, name=f"s2_{t}", tag="s2")
                nc.vector.reciprocal(s2[:], sq[:])
                x1n = []
                for i in range(NCT):
                    xni = x1np.tile([NP, L2], BF16, name=f"x1n_{t}_{i}", tag="x1n")
                    nc.vector.tensor_tensor(xni[:], x1[i][:], s2[:], OP.mult)
                    x1n.append(xni)

                # grow proj: gate2 rows [0:2D), v rows [2D:4D)
                gv = []
                for hm in range(2 * NCT):
                    psg = pmm2.tile([NP, L2], F32, name=f"g2_ps_{t}_{hm}", tag="mm2")
                    for k in range(NCT):
                        lhs = wgr_sb[:, k * HID + hm * NP: k * HID + (hm + 1) * NP]
                        nc.tensor.matmul(psg[:], lhs, x1n[k][:],
                                         start=(k == 0), stop=(k == NCT - 1))
                    psv = pmm2.tile([NP, L2], F32, name=f"v_ps_{t}_{hm}", tag="mm2")
                    for k in range(NCT):
                        lhs = wgr_sb[:, k * HID + H2 + hm * NP:
                                     k * HID + H2 + (hm + 1) * NP]
                        nc.tensor.matmul(psv[:], lhs, x1n[k][:],
                                         start=(k == 0), stop=(k == NCT - 1))
                    t2 = sp2.tile([NP, L2], BF16, name=f"t2_{t}_{hm}", tag="t2")
                    if gelu_approx:
                        sg2 = sp2.tile([NP, L2], F32, name=f"sg2_{t}_{hm}", tag="sg2")
                        nc.scalar.activation(sg2[:], psg[:], AF.Sigmoid,
                                             scale=1.702)
                        nc.vector.tensor_tensor(t2[:], psg[:], sg2[:], OP.mult)
                    else:
                        nc.scalar.activation(t2[:], psg[:], AF.Gelu)
                    gvi = gvp.tile([NP, L2], BF16, name=f"gv_{t}_{hm}", tag="gv")
                    nc.vector.tensor_tensor(gvi[:], t2[:], psv[:], OP.mult)
                    gv.append(gvi)

                # shrink proj + residual -> out
                for m in range(NCT):
                    ps = pmm2.tile([NP, L2], F32, name=f"s_ps_{t}_{m}", tag="mm2")
                    for k in range(2 * NCT):
                        lhs = wsh_sb[:, k * D + m * NP: k * D + (m + 1) * NP]
                        nc.tensor.matmul(ps[:], lhs, gv[k][:],
                                         start=(k == 0), stop=(k == 2 * NCT - 1))
                    om = op_.tile([NP, L2], F32, name=f"out_{t}_{m}", tag="out")
                    nc.vector.tensor_tensor(om[:], ps[:], x1[m][:], OP.add)
                    nc.sync.dma_start(
                        out_d.ap()[m * NP:(m + 1) * NP, t * L2:(t + 1) * L2], om[:])

    nc.compile()
    return nc


def host_prepare(inputs, T_core, n_cores=N_CORES):
    """Build per-core in_maps from full inputs."""
    x = np.asarray(inputs["x"], np.float32)            # [B, T, D]
    B, T, _ = x.shape
    halves = n_cores // B
    assert T == halves * T_core

    gam1 = np.asarray(inputs["hawk_norm_gamma"], np.float32)
    gam2 = np.asarray(inputs["gmlp_norm_gamma"], np.float32)
    scale1 = gam1 * np.sqrt(D)
    scale2 = gam2 * np.sqrt(D)

    wi = (np.asarray(inputs["input_w"], np.float32) * scale1[None, :]).T
    wg = np.asarray(inputs["gates_w"], np.float32).T
    wo = np.asarray(inputs["output_w"], np.float32).T
    wgr = (np.asarray(inputs["grow_w"], np.float32) * scale2[None, :]).T
    wsh = np.asarray(inputs["shrink_w"], np.float32).T

    fb = np.asarray(inputs["forget_base"], np.float64)
    msp = (-8.0 * np.log1p(np.exp(fb))).astype(np.float32)

    def chan_layout(v):  # [D] -> [128, 8] with [p, i] = v[128*i + p]
        return np.ascontiguousarray(v.reshape(NCT, NP).T)

    gb = np.asarray(inputs["gates_b"], np.float32)
    cw = np.asarray(inputs["conv_w"], np.float32)[:, 0, :]   # [D, K]
    cb = np.asarray(inputs["conv_b"], np.float32)

    shared = {
        "wi": wi.astype(_BF), "wg": wg.astype(_BF), "wo": wo.astype(_BF),
        "wgr": wgr.astype(_BF), "wsh": wsh.astype(_BF),
        "msp": chan_layout(msp), "msp2": chan_layout(2.0 * msp),
        "gbf": chan_layout(gb[:D]), "gbi": chan_layout(gb[D:]),
        "cw": np.concatenate([chan_layout(cw[:, k]) for k in range(KCONV)],
                             axis=1),
        "cb": chan_layout(cb),
    }
    in_maps = []
    for core in range(n_cores):
        b, h = core // halves, core % halves
        xf = np.zeros((D, 3 + T_core), np.float32)
        xf[:, 3:] = x[b, h * T_core:(h + 1) * T_core, :].T
        if h > 0:
            xf[:, 0:3] = x[b, h * T_core - 3:h * T_core, :].T
        m = dict(shared)
        m["x"] = xf
        m["cmask"] = np.full((NP, 1), 1.0 if h > 0 else 0.0, np.float32)
        in_maps.append(m)
    return in_maps


def assemble_output(results, B, T, T_core, n_cores=N_CORES):
    halves = n_cores // B
    out = np.empty((B, T, D), np.float32)
    for core in range(n_cores):
        b, h = core // halves, core % halves
        out[b, h * T_core:(h + 1) * T_core, :] = results[core]["out"].T
    return out


_PROG_CACHE = {}


def kernel(**inputs) -> np.ndarray:
    x = np.asarray(inputs["x"])
    B, T, _ = x.shape
    T_core = T * B // N_CORES
    L = 512 if T_core % 512 == 0 else T_core // 4
    key = (T_core, L)
    if key not in _PROG_CACHE:
        _PROG_CACHE[key] = build_program(T_core, L)
    nc = _PROG_CACHE[key]
    in_maps = host_prepare(inputs, T_core)
    res = run_bass_kernel_spmd(nc, in_maps, list(range(N_CORES)))
    return assemble_output(res.results, B, T, T_core)

